# revision 6
# baseline (speedup 1.0000x reference)
"""Single-head attention (B=4, L=2048, C=512) on 8 NeuronCores.

Sharding: data-parallel over batch (4) x query-halves (2) = 8 shards.
Each core gets a [1024, 512] query slice plus the full [2048, 512] K/V
for its batch, and computes the complete attention output for its rows.
No cross-core communication is needed.

Per-core dataflow (all matmuls on the PE array in fp32r):
  - inputs are DMA'd with a 32x32 block-swizzled access pattern and
    fixed up with the DVE 32x32 stream-transpose, giving X^T in SBUF
    ([C-chunk on partitions, L free]) without using PE transposes.
  - Q^T = Wq^T X^T + bq (bias via per-partition activation), same K^T.
  - V   = (X_v^T)^T Wv in natural [L, C] layout (bias folded later).
  - S^T = K Q^T  -> P^T = exp(s * S^T)   (no row-max subtraction:
    scores are ~N(0,1), |s*S| < ~7, exp is fp32-safe).
  - d   = 1^T P^T (row sums as a [1, Lq] row via K=128 matmul with ones)
  - O^T = V^T P^T + bv x d   (bv outer product folds the V bias)
  - Y   = (O^T)^T Wo + d x bo, then scaled by 1/d per row on PSUM->SBUF
    copy (reciprocal row scattered to column layout by a tiny DMA).
"""

import sys
import numpy as np

for _p in ("/opt/trn_rl_repo",):
    if _p not in sys.path:
        sys.path.append(_p)

import concourse.bass as bass  # noqa: E402
import concourse.tile as tile  # noqa: E402
from concourse import bacc, mybir  # noqa: E402

F32 = mybir.dt.float32
F32R = mybir.dt.float32r
AFT = mybir.ActivationFunctionType

B, L, C = 4, 2048, 512
N_CORES = 8
LQ = (B * L) // N_CORES  # 1024 query rows per core
LK = L  # 2048 key rows per core
SM_SCALE = 1.0 / float(np.sqrt(C))

NKC = C // 128  # 4 contraction chunks
NQB = LQ // 512  # 2 query blocks
NKT = LK // 128  # 16 key tiles


def build_nc():
    """Build the per-core Bass module (identical SPMD program on 8 cores)."""
    nc = bacc.Bacc("TRN2", target_bir_lowering=False, debug=False)

    q_d = nc.dram_tensor("query", [LQ, C], F32R, kind="ExternalInput").ap()
    k_d = nc.dram_tensor("key", [LK, C], F32R, kind="ExternalInput").ap()
    v_d = nc.dram_tensor("value", [LK, C], F32R, kind="ExternalInput").ap()
    w_d = {
        w: nc.dram_tensor(w, [C, C], F32R, kind="ExternalInput").ap()
        for w in ("Wq", "Wk", "Wv", "Wo")
    }
    b_d = {
        b: nc.dram_tensor(b, [C], F32R if b in ("bv", "bo") else F32,
                          kind="ExternalInput").ap()
        for b in ("bq", "bk", "bv", "bo")
    }
    out_d = nc.dram_tensor("out", [LQ, C], F32, kind="ExternalOutput").ap()

    with tile.TileContext(nc) as tc:
        with (
            tc.tile_pool(name="wpool", bufs=1) as wpool,
            tc.tile_pool(name="vpool", bufs=1) as vpool,
            tc.tile_pool(name="ktp", bufs=1) as kt_pool,
            tc.tile_pool(name="qtp", bufs=1) as qt_pool,
        ):
            # ---- long-lived small constants ---------------------------
            w_sb = {}

            def load_w(wn, pool):
                w_sb[wn] = []
                for kc in range(NKC):
                    wt = pool.tile([128, C], F32R, name=f"{wn}_{kc}", tag=f"{wn}_{kc}")
                    nc.sync.dma_start(wt[:], w_d[wn][kc * 128 : (kc + 1) * 128, :])
                    w_sb[wn].append(wt)

            load_w("Wo", wpool)
            bq_col = wpool.tile([128, NKC], F32, name="bq_col", tag="bq_col")
            bk_col = wpool.tile([128, NKC], F32, name="bk_col", tag="bk_col")
            for col, src in ((bq_col, b_d["bq"]), (bk_col, b_d["bk"])):
                for kc in range(NKC):
                    nc.sync.dma_start(
                        col[:, kc : kc + 1], src[kc * 128 : (kc + 1) * 128]
                    )
            bv_row = wpool.tile([1, C], F32R, name="bv_row", tag="bv_row")
            bo_row = wpool.tile([1, C], F32R, name="bo_row", tag="bo_row")
            nc.sync.dma_start(bv_row[0:1, :], b_d["bv"])
            nc.sync.dma_start(bo_row[0:1, :], b_d["bo"])
            ones_col = wpool.tile([128, 1], F32R, name="ones_col", tag="ones_col")
            nc.gpsimd.memset(ones_col[:].bitcast(mybir.dt.uint32), 0x3F800000)

            # ---- transposed-load + projection phase -------------------
            # Swizzled DRAM view: l = 512 s + 128 lt + 32 b + pp,
            # c = 128 k + 32 a + ff. Staging tile [128=(a pp), 512=(lt b
            # ff)] holds 32x32 blocks placed transposed with natural
            # content, so one DVE stream-transpose yields
            # X^T[128k:+128, 512s:+512]. One DMA per (k, a) keeps the
            # DRAM access pattern within 3 dims.
            v_sb = [
                vpool.tile([128, C], F32R, name=f"v{m}", tag=f"v{m}")
                for m in range(NKT)
            ]

            with (
                tc.tile_pool(name="wproj", bufs=1) as wproj,
                tc.tile_pool(name="stg", bufs=3) as stg_pool,
                tc.tile_pool(name="xts", bufs=2) as xts_pool,
                tc.tile_pool(name="psA", bufs=4, space="PSUM") as ps_pool,
            ):
                load_w("Wv", wproj)
                load_w("Wk", wproj)
                load_w("Wq", wproj)

                def load_xt_slab(view, s):
                    xt = [
                        xts_pool.tile([128, 512], F32R, name=f"xt{kc}", tag=f"xt{kc}")
                        for kc in range(NKC)
                    ]
                    for kc in range(NKC):
                        stg = stg_pool.tile([128, 512], F32R, name="stg", tag="stg")
                        for a in range(4):
                            nc.sync.dma_start(
                                stg[32 * a : 32 * a + 32, :], view[s, kc, a]
                            )
                        xtf = stg_pool.tile([128, 512], F32, name="xtf", tag="xtf")
                        nc.vector.transpose(xtf[:], stg[:].bitcast(F32))
                        nc.vector.tensor_copy(xt[kc][:], xtf[:])
                    return xt

                def swizzled(dram):
                    return dram.rearrange(
                        "(s lt b pp) (k a ff) -> s k a pp lt b ff",
                        lt=4, b=4, pp=32, k=NKC, a=4, ff=32,
                    )

                # V in natural [LK, C] layout: V = Xv @ Wv (bias folded
                # into O^T later via bv x d).
                vview = swizzled(v_d)
                for s in range(LK // 512):
                    xt = load_xt_slab(vview, s)
                    for mt in range(4):
                        m = 4 * s + mt
                        ps = ps_pool.tile([128, 512], F32, name="ps", tag="ps")
                        for kc in range(NKC):
                            nc.tensor.matmul(
                                ps[:],
                                (xt[kc][:, mt * 128 : (mt + 1) * 128]),
                                (w_sb["Wv"][kc][:]),
                                start=(kc == 0),
                                stop=(kc == NKC - 1),
                            )
                        nc.vector.tensor_copy(v_sb[m][:], ps[:])

                # K^T and Q^T in [C_out, L] layout with per-partition bias.
                def project_T(dram, wn, b_col, Lx, pool, basename):
                    res = [
                        pool.tile(
                            [128, Lx], F32R, name=f"{basename}{j}", tag=f"{basename}{j}"
                        )
                        for j in range(NKC)
                    ]
                    view = swizzled(dram)
                    for s in range(Lx // 512):
                        xt = load_xt_slab(view, s)
                        for j in range(NKC):
                            ps = ps_pool.tile([128, 512], F32, name="ps", tag="ps")
                            for kc in range(NKC):
                                nc.tensor.matmul(
                                    ps[:],
                                    (w_sb[wn][kc][:, j * 128 : (j + 1) * 128]),
                                    (xt[kc][:]),
                                    start=(kc == 0),
                                    stop=(kc == NKC - 1),
                                )
                            nc.scalar.activation(
                                res[j][:, s * 512 : (s + 1) * 512],
                                ps[:],
                                AFT.Identity,
                                bias=b_col[:, j : j + 1],
                            )
                    return res

                kt = project_T(k_d, "Wk", bk_col, LK, kt_pool, "kt")
                qt = project_T(q_d, "Wq", bq_col, LQ, qt_pool, "qt")

            # ---- attention phase --------------------------------------
            with (
                tc.tile_pool(name="ptp", bufs=1) as pt_pool,
                tc.tile_pool(name="otp", bufs=1) as ot_pool,
                tc.tile_pool(name="ypool", bufs=2) as y_pool,
                tc.tile_pool(name="dpool", bufs=1) as d_pool,
                tc.tile_pool(name="psS", bufs=3, space="PSUM") as ps_s,
                tc.tile_pool(name="psD", bufs=1, space="PSUM") as ps_d,
                tc.tile_pool(name="psO", bufs=2, space="PSUM") as ps_o,
                tc.tile_pool(name="psY", bufs=2, space="PSUM") as ps_y,
            ):
                for blk in range(NQB):
                    qs = slice(blk * 512, (blk + 1) * 512)
                    # P^T = exp(s * S^T) tile-by-tile over LK
                    pt = [
                        pt_pool.tile([128, 512], F32R, name=f"pt{lk}", tag=f"pt{lk}")
                        for lk in range(NKT)
                    ]
                    for lk in range(NKT):
                        ps = ps_s.tile([128, 512], F32, name="ps_s", tag="s")
                        for kc in range(NKC):
                            nc.tensor.matmul(
                                ps[:],
                                (kt[kc][:, lk * 128 : (lk + 1) * 128]),
                                (qt[kc][:, qs]),
                                start=(kc == 0),
                                stop=(kc == NKC - 1),
                            )
                        nc.scalar.activation(
                            pt[lk][:], ps[:], AFT.Exp, scale=SM_SCALE
                        )

                    # d = 1^T P^T : [1, 512] row of softmax denominators
                    psd = ps_d.tile([1, 512], F32, name="psd", tag="d")
                    for lk in range(NKT):
                        nc.tensor.matmul(
                            psd[:],
                            (ones_col[:]),
                            (pt[lk][:]),
                            start=(lk == 0),
                            stop=(lk == NKT - 1),
                        )
                    d_row = d_pool.tile([1, 512], F32R, name=f"d_row{blk}", tag=f"d_row{blk}")
                    nc.vector.tensor_copy(d_row[:], psd[:])
                    # scatter the row to column layout, then reciprocal
                    d_col = d_pool.tile([128, 4], F32R, name=f"d_col{blk}", tag=f"d_col{blk}")
                    for mm in range(4):
                        nc.sync.dma_start(
                            d_col[:, mm : mm + 1], d_row[0:1, mm * 128 : (mm + 1) * 128]
                        )
                    d_rcp = d_pool.tile([128, 4], F32, name=f"d_rcp{blk}", tag=f"d_rcp{blk}")
                    nc.vector.reciprocal(d_rcp[:], d_col[:].bitcast(F32))

                    # O^T = V^T P^T + bv x d
                    ot = [
                        ot_pool.tile([128, 512], F32R, name=f"ot{cc}", tag=f"ot{cc}")
                        for cc in range(NKC)
                    ]
                    for cc in range(NKC):
                        pso = ps_o.tile([128, 512], F32, name="ps_o", tag="o")
                        for lk in range(NKT):
                            nc.tensor.matmul(
                                pso[:],
                                (v_sb[lk][:, cc * 128 : (cc + 1) * 128]),
                                (pt[lk][:]),
                                start=(lk == 0),
                                stop=False,
                            )
                        nc.tensor.matmul(
                            pso[:],
                            (bv_row[0:1, cc * 128 : (cc + 1) * 128]),
                            (d_row[:]),
                            start=False,
                            stop=True,
                        )
                        nc.vector.tensor_copy(ot[cc][:], pso[:])

                    # Y = (O^T)^T Wo + d x bo, then row-scale by 1/d
                    for mm in range(4):
                        psy = ps_y.tile([128, 512], F32, name="ps_y", tag="y")
                        for cc in range(NKC):
                            nc.tensor.matmul(
                                psy[:],
                                (ot[cc][:, mm * 128 : (mm + 1) * 128]),
                                (w_sb["Wo"][cc][:]),
                                start=(cc == 0),
                                stop=False,
                            )
                        nc.tensor.matmul(
                            psy[:],
                            (d_row[0:1, mm * 128 : (mm + 1) * 128]),
                            (bo_row[:]),
                            start=False,
                            stop=True,
                        )
                        y_sb = y_pool.tile([128, C], F32, name="y_sb", tag="y_sb")
                        nc.scalar.activation(
                            y_sb[:], psy[:], AFT.Copy, scale=d_rcp[:, mm : mm + 1]
                        )
                        row0 = blk * 512 + mm * 128
                        nc.sync.dma_start(out_d[row0 : row0 + 128, :], y_sb[:])

    nc.compile()
    return nc


_NC_CACHE = None


def get_nc():
    global _NC_CACHE
    if _NC_CACHE is None:
        _NC_CACHE = build_nc()
    return _NC_CACHE


def make_in_maps(inputs):
    """Shard the full inputs into 8 per-core input maps."""
    full_q = np.asarray(inputs["query"], dtype=np.float32)
    full_k = np.asarray(inputs["key"], dtype=np.float32)
    full_v = np.asarray(inputs["value"], dtype=np.float32)
    shared = {
        n: np.ascontiguousarray(np.asarray(inputs[n], dtype=np.float32))
        for n in ("Wq", "bq", "Wk", "bk", "Wv", "bv", "Wo", "bo")
    }
    in_maps = []
    for c in range(N_CORES):
        b, h = divmod(c, 2)
        m = {
            "query": np.ascontiguousarray(full_q[b, h * LQ : (h + 1) * LQ]),
            "key": np.ascontiguousarray(full_k[b]),
            "value": np.ascontiguousarray(full_v[b]),
        }
        m.update(shared)
        in_maps.append(m)
    return in_maps


def kernel(**inputs) -> np.ndarray:
    from concourse.bass_utils import run_bass_kernel_spmd

    nc = get_nc()
    in_maps = make_in_maps(inputs)
    res = run_bass_kernel_spmd(nc, in_maps, list(range(N_CORES)))
    out = np.empty((B, L, C), dtype=np.float32)
    for c in range(N_CORES):
        b, h = divmod(c, 2)
        out[b, h * LQ : (h + 1) * LQ] = res.results[c]["out"]
    return out


# revision 7
# speedup vs baseline: 1.0050x; 1.0050x over previous
"""Single-head attention (B=4, L=2048, C=512) on 8 NeuronCores.

Sharding: data-parallel over batch (4) x query-halves (2) = 8 shards.
Each core gets a [1024, 512] query slice plus the full [2048, 512] K/V
for its batch, and computes the complete attention output for its rows.
No cross-core communication is needed.

Per-core dataflow (all matmuls on the PE array in fp32r):
  - inputs are DMA'd with a 32x32 block-swizzled access pattern and
    fixed up with the DVE 32x32 stream-transpose, giving X^T in SBUF
    ([C-chunk on partitions, L free]) without using PE transposes.
  - Q^T = Wq^T X^T + bq (bias via per-partition activation), same K^T.
  - V   = (X_v^T)^T Wv in natural [L, C] layout (bias folded later).
  - S^T = K Q^T  -> P^T = exp(s * S^T)   (no row-max subtraction:
    scores are ~N(0,1), |s*S| < ~7, exp is fp32-safe).
  - d   = 1^T P^T (row sums as a [1, Lq] row via K=128 matmul with ones)
  - O^T = V^T P^T + bv x d   (bv outer product folds the V bias)
  - Y   = (O^T)^T Wo + d x bo, then scaled by 1/d per row on PSUM->SBUF
    copy (reciprocal row scattered to column layout by a tiny DMA).
"""

import sys
import numpy as np

for _p in ("/opt/trn_rl_repo",):
    if _p not in sys.path:
        sys.path.append(_p)

import concourse.bass as bass  # noqa: E402
import concourse.tile as tile  # noqa: E402
from concourse import bacc, mybir  # noqa: E402

F32 = mybir.dt.float32
F32R = mybir.dt.float32r
AFT = mybir.ActivationFunctionType

B, L, C = 4, 2048, 512
N_CORES = 8
LQ = (B * L) // N_CORES  # 1024 query rows per core
LK = L  # 2048 key rows per core
SM_SCALE = 1.0 / float(np.sqrt(C))

NKC = C // 128  # 4 contraction chunks
NQB = LQ // 512  # 2 query blocks
NKT = LK // 128  # 16 key tiles


def build_nc():
    """Build the per-core Bass module (identical SPMD program on 8 cores)."""
    nc = bacc.Bacc("TRN2", target_bir_lowering=False, debug=False)

    q_d = nc.dram_tensor("query", [LQ, C], F32R, kind="ExternalInput").ap()
    k_d = nc.dram_tensor("key", [LK, C], F32R, kind="ExternalInput").ap()
    v_d = nc.dram_tensor("value", [LK, C], F32R, kind="ExternalInput").ap()
    w_d = {
        w: nc.dram_tensor(w, [C, C], F32R, kind="ExternalInput").ap()
        for w in ("Wq", "Wk", "Wv", "Wo")
    }
    b_d = {
        b: nc.dram_tensor(b, [C], F32R if b in ("bv", "bo") else F32,
                          kind="ExternalInput").ap()
        for b in ("bq", "bk", "bv", "bo")
    }
    out_d = nc.dram_tensor("out", [LQ, C], F32, kind="ExternalOutput").ap()

    with tile.TileContext(nc) as tc:
        with (
            tc.tile_pool(name="wpool", bufs=1) as wpool,
            tc.tile_pool(name="vpool", bufs=1) as vpool,
            tc.tile_pool(name="ktp", bufs=1) as kt_pool,
            tc.tile_pool(name="qtp", bufs=1) as qt_pool,
        ):
            # ---- long-lived small constants ---------------------------
            w_sb = {}

            def load_w(wn, pool):
                w_sb[wn] = []
                for kc in range(NKC):
                    wt = pool.tile([128, C], F32R, name=f"{wn}_{kc}", tag=f"{wn}_{kc}")
                    nc.gpsimd.dma_start(wt[:], w_d[wn][kc * 128 : (kc + 1) * 128, :])
                    w_sb[wn].append(wt)

            load_w("Wo", wpool)
            bq_col = wpool.tile([128, NKC], F32, name="bq_col", tag="bq_col")
            bk_col = wpool.tile([128, NKC], F32, name="bk_col", tag="bk_col")
            for col, src in ((bq_col, b_d["bq"]), (bk_col, b_d["bk"])):
                for kc in range(NKC):
                    nc.gpsimd.dma_start(
                        col[:, kc : kc + 1], src[kc * 128 : (kc + 1) * 128]
                    )
            bv_row = wpool.tile([1, C], F32R, name="bv_row", tag="bv_row")
            bo_row = wpool.tile([1, C], F32R, name="bo_row", tag="bo_row")
            nc.gpsimd.dma_start(bv_row[0:1, :], b_d["bv"])
            nc.gpsimd.dma_start(bo_row[0:1, :], b_d["bo"])
            ones_col = wpool.tile([128, 1], F32R, name="ones_col", tag="ones_col")
            nc.gpsimd.memset(ones_col[:].bitcast(mybir.dt.uint32), 0x3F800000)

            # ---- transposed-load + projection phase -------------------
            # Swizzled DRAM view: l = 512 s + 128 lt + 32 b + pp,
            # c = 128 k + 32 a + ff. Staging tile [128=(a pp), 512=(lt b
            # ff)] holds 32x32 blocks placed transposed with natural
            # content, so one DVE stream-transpose yields
            # X^T[128k:+128, 512s:+512]. One DMA per (k, a) keeps the
            # DRAM access pattern within 3 dims.
            v_sb = [
                vpool.tile([128, C], F32R, name=f"v{m}", tag=f"v{m}")
                for m in range(NKT)
            ]

            with (
                tc.tile_pool(name="wproj", bufs=1) as wproj,
                tc.tile_pool(name="stg", bufs=4) as stg_pool,
                tc.tile_pool(name="xts", bufs=3) as xts_pool,
                tc.tile_pool(name="psA", bufs=6, space="PSUM") as ps_pool,
            ):
                load_w("Wv", wproj)
                load_w("Wk", wproj)
                load_w("Wq", wproj)

                def load_xt_slab(view, s):
                    xt = [
                        xts_pool.tile([128, 512], F32R, name=f"xt{kc}", tag=f"xt{kc}")
                        for kc in range(NKC)
                    ]
                    for kc in range(NKC):
                        stg = stg_pool.tile([128, 512], F32R, name="stg", tag="stg")
                        for a in range(4):
                            eng = nc.sync if (kc * 4 + a) % 2 == 0 else nc.scalar
                            eng.dma_start(
                                stg[32 * a : 32 * a + 32, :], view[s, kc, a]
                            )
                        xtf = stg_pool.tile([128, 512], F32, name="xtf", tag="xtf")
                        nc.vector.transpose(xtf[:], stg[:].bitcast(F32))
                        nc.scalar.activation(xt[kc][:], xtf[:], AFT.Identity)
                    return xt

                def swizzled(dram):
                    return dram.rearrange(
                        "(s lt b pp) (k a ff) -> s k a pp lt b ff",
                        lt=4, b=4, pp=32, k=NKC, a=4, ff=32,
                    )

                # K^T and Q^T in [C_out, L] layout with per-partition bias.
                def project_T(dram, wn, b_col, Lx, pool, basename):
                    res = [
                        pool.tile(
                            [128, Lx], F32R, name=f"{basename}{j}", tag=f"{basename}{j}"
                        )
                        for j in range(NKC)
                    ]
                    view = swizzled(dram)
                    for s in range(Lx // 512):
                        xt = load_xt_slab(view, s)
                        for j in range(NKC):
                            ps = ps_pool.tile([128, 512], F32, name="ps", tag="ps")
                            for kc in range(NKC):
                                nc.tensor.matmul(
                                    ps[:],
                                    (w_sb[wn][kc][:, j * 128 : (j + 1) * 128]),
                                    (xt[kc][:]),
                                    start=(kc == 0),
                                    stop=(kc == NKC - 1),
                                )
                            nc.scalar.activation(
                                res[j][:, s * 512 : (s + 1) * 512],
                                ps[:],
                                AFT.Identity,
                                bias=b_col[:, j : j + 1],
                            )
                    return res

                kt = project_T(k_d, "Wk", bk_col, LK, kt_pool, "kt")
                qt = project_T(q_d, "Wq", bq_col, LQ, qt_pool, "qt")

                # V in natural [LK, C] layout: V = Xv @ Wv (bias folded
                # into O^T later via bv x d).
                vview = swizzled(v_d)
                for s in range(LK // 512):
                    xt = load_xt_slab(vview, s)
                    for mt in range(4):
                        m = 4 * s + mt
                        ps = ps_pool.tile([128, 512], F32, name="ps", tag="ps")
                        for kc in range(NKC):
                            nc.tensor.matmul(
                                ps[:],
                                (xt[kc][:, mt * 128 : (mt + 1) * 128]),
                                (w_sb["Wv"][kc][:]),
                                start=(kc == 0),
                                stop=(kc == NKC - 1),
                            )
                        nc.vector.tensor_copy(v_sb[m][:], ps[:])

            # ---- attention phase --------------------------------------
            with (
                tc.tile_pool(name="ptp", bufs=1) as pt_pool,
                tc.tile_pool(name="otp", bufs=1) as ot_pool,
                tc.tile_pool(name="ypool", bufs=2) as y_pool,
                tc.tile_pool(name="dpool", bufs=1) as d_pool,
                tc.tile_pool(name="psS", bufs=3, space="PSUM") as ps_s,
                tc.tile_pool(name="psD", bufs=1, space="PSUM") as ps_d,
                tc.tile_pool(name="psO", bufs=2, space="PSUM") as ps_o,
                tc.tile_pool(name="psY", bufs=2, space="PSUM") as ps_y,
            ):
                for blk in range(NQB):
                    qs = slice(blk * 512, (blk + 1) * 512)
                    # P^T = exp(s * S^T) tile-by-tile over LK
                    pt = [
                        pt_pool.tile([128, 512], F32R, name=f"pt{lk}", tag=f"pt{lk}")
                        for lk in range(NKT)
                    ]
                    for lk in range(NKT):
                        ps = ps_s.tile([128, 512], F32, name="ps_s", tag="s")
                        for kc in range(NKC):
                            nc.tensor.matmul(
                                ps[:],
                                (kt[kc][:, lk * 128 : (lk + 1) * 128]),
                                (qt[kc][:, qs]),
                                start=(kc == 0),
                                stop=(kc == NKC - 1),
                            )
                        nc.scalar.activation(
                            pt[lk][:], ps[:], AFT.Exp, scale=SM_SCALE
                        )

                    # d = 1^T P^T : [1, 512] row of softmax denominators
                    psd = ps_d.tile([1, 512], F32, name="psd", tag="d")
                    for lk in range(NKT):
                        nc.tensor.matmul(
                            psd[:],
                            (ones_col[:]),
                            (pt[lk][:]),
                            start=(lk == 0),
                            stop=(lk == NKT - 1),
                        )
                    d_row = d_pool.tile([1, 512], F32R, name=f"d_row{blk}", tag=f"d_row{blk}")
                    nc.vector.tensor_copy(d_row[:], psd[:])
                    # scatter the row to column layout, then reciprocal
                    d_col = d_pool.tile([128, 4], F32R, name=f"d_col{blk}", tag=f"d_col{blk}")
                    for mm in range(4):
                        nc.sync.dma_start(
                            d_col[:, mm : mm + 1], d_row[0:1, mm * 128 : (mm + 1) * 128]
                        )
                    d_rcp = d_pool.tile([128, 4], F32, name=f"d_rcp{blk}", tag=f"d_rcp{blk}")
                    nc.vector.reciprocal(d_rcp[:], d_col[:].bitcast(F32))

                    # O^T = V^T P^T + bv x d
                    ot = [
                        ot_pool.tile([128, 512], F32R, name=f"ot{cc}", tag=f"ot{cc}")
                        for cc in range(NKC)
                    ]
                    for cc in range(NKC):
                        pso = ps_o.tile([128, 512], F32, name="ps_o", tag="o")
                        for lk in range(NKT):
                            nc.tensor.matmul(
                                pso[:],
                                (v_sb[lk][:, cc * 128 : (cc + 1) * 128]),
                                (pt[lk][:]),
                                start=(lk == 0),
                                stop=False,
                            )
                        nc.tensor.matmul(
                            pso[:],
                            (bv_row[0:1, cc * 128 : (cc + 1) * 128]),
                            (d_row[:]),
                            start=False,
                            stop=True,
                        )
                        nc.vector.tensor_copy(ot[cc][:], pso[:])

                    # Y = (O^T)^T Wo + d x bo, then row-scale by 1/d
                    for mm in range(4):
                        psy = ps_y.tile([128, 512], F32, name="ps_y", tag="y")
                        for cc in range(NKC):
                            nc.tensor.matmul(
                                psy[:],
                                (ot[cc][:, mm * 128 : (mm + 1) * 128]),
                                (w_sb["Wo"][cc][:]),
                                start=(cc == 0),
                                stop=False,
                            )
                        nc.tensor.matmul(
                            psy[:],
                            (d_row[0:1, mm * 128 : (mm + 1) * 128]),
                            (bo_row[:]),
                            start=False,
                            stop=True,
                        )
                        y_sb = y_pool.tile([128, C], F32, name="y_sb", tag="y_sb")
                        nc.scalar.activation(
                            y_sb[:], psy[:], AFT.Copy, scale=d_rcp[:, mm : mm + 1]
                        )
                        row0 = blk * 512 + mm * 128
                        nc.sync.dma_start(out_d[row0 : row0 + 128, :], y_sb[:])

    nc.compile()
    return nc


_NC_CACHE = None


def get_nc():
    global _NC_CACHE
    if _NC_CACHE is None:
        _NC_CACHE = build_nc()
    return _NC_CACHE


def make_in_maps(inputs):
    """Shard the full inputs into 8 per-core input maps."""
    full_q = np.asarray(inputs["query"], dtype=np.float32)
    full_k = np.asarray(inputs["key"], dtype=np.float32)
    full_v = np.asarray(inputs["value"], dtype=np.float32)
    shared = {
        n: np.ascontiguousarray(np.asarray(inputs[n], dtype=np.float32))
        for n in ("Wq", "bq", "Wk", "bk", "Wv", "bv", "Wo", "bo")
    }
    in_maps = []
    for c in range(N_CORES):
        b, h = divmod(c, 2)
        m = {
            "query": np.ascontiguousarray(full_q[b, h * LQ : (h + 1) * LQ]),
            "key": np.ascontiguousarray(full_k[b]),
            "value": np.ascontiguousarray(full_v[b]),
        }
        m.update(shared)
        in_maps.append(m)
    return in_maps


def kernel(**inputs) -> np.ndarray:
    from concourse.bass_utils import run_bass_kernel_spmd

    nc = get_nc()
    in_maps = make_in_maps(inputs)
    res = run_bass_kernel_spmd(nc, in_maps, list(range(N_CORES)))
    out = np.empty((B, L, C), dtype=np.float32)
    for c in range(N_CORES):
        b, h = divmod(c, 2)
        out[b, h * LQ : (h + 1) * LQ] = res.results[c]["out"]
    return out


# revision 9
# speedup vs baseline: 1.0669x; 1.0617x over previous
"""Single-head attention (B=4, L=2048, C=512) on 8 NeuronCores.

Sharding: data-parallel over batch (4) x query-halves (2) = 8 shards.
Each core gets a [1024, 512] query slice plus the full [2048, 512] K/V
for its batch, and computes the complete attention output for its rows.
No cross-core communication is needed.

Per-core dataflow (all matmuls on the PE array in fp32r):
  - inputs are DMA'd with a 32x32 block-swizzled access pattern and
    fixed up with the DVE 32x32 stream-transpose, giving X^T in SBUF
    ([C-chunk on partitions, L free]) without using PE transposes.
  - Q^T = Wq^T X^T + bq (bias via per-partition activation), same K^T.
  - V   = (X_v^T)^T Wv in natural [L, C] layout (bias folded later).
  - S^T = K Q^T  -> P^T = exp(s * S^T)   (no row-max subtraction:
    scores are ~N(0,1), |s*S| < ~7, exp is fp32-safe).
  - d   = 1^T P^T (row sums as a [1, Lq] row via K=128 matmul with ones)
  - O^T = V^T P^T + bv x d   (bv outer product folds the V bias)
  - Y   = (O^T)^T Wo + d x bo, then scaled by 1/d per row on PSUM->SBUF
    copy (reciprocal row scattered to column layout by a tiny DMA).
"""

import sys
import numpy as np

for _p in ("/opt/trn_rl_repo",):
    if _p not in sys.path:
        sys.path.append(_p)

import concourse.bass as bass  # noqa: E402
import concourse.tile as tile  # noqa: E402
from concourse import bacc, mybir  # noqa: E402

F32 = mybir.dt.float32
F32R = mybir.dt.float32r
AFT = mybir.ActivationFunctionType

B, L, C = 4, 2048, 512
N_CORES = 8
LQ = (B * L) // N_CORES  # 1024 query rows per core
LK = L  # 2048 key rows per core
SM_SCALE = 1.0 / float(np.sqrt(C))

NKC = C // 128  # 4 contraction chunks
NQB = LQ // 512  # 2 query blocks
NKT = LK // 128  # 16 key tiles


def build_nc():
    """Build the per-core Bass module (identical SPMD program on 8 cores)."""
    nc = bacc.Bacc("TRN2", target_bir_lowering=False, debug=False)

    q_d = nc.dram_tensor("query", [LQ, C], F32R, kind="ExternalInput").ap()
    k_d = nc.dram_tensor("key", [LK, C], F32R, kind="ExternalInput").ap()
    v_d = nc.dram_tensor("value", [LK, C], F32R, kind="ExternalInput").ap()
    w_d = {
        w: nc.dram_tensor(w, [C, C], F32R, kind="ExternalInput").ap()
        for w in ("Wq", "Wk", "Wv", "Wo")
    }
    b_d = {
        b: nc.dram_tensor(b, [C], F32R if b in ("bv", "bo") else F32,
                          kind="ExternalInput").ap()
        for b in ("bq", "bk", "bv", "bo")
    }
    out_d = nc.dram_tensor("out", [LQ, C], F32, kind="ExternalOutput").ap()

    with tile.TileContext(nc) as tc:
        with (
            tc.tile_pool(name="wpool", bufs=1) as wpool,
            tc.tile_pool(name="vpool", bufs=1) as vpool,
            tc.tile_pool(name="ktp", bufs=1) as kt_pool,
            tc.tile_pool(name="qtp", bufs=1) as qt_pool,
        ):
            # ---- long-lived small constants ---------------------------
            w_sb = {}

            def load_w(wn, pool):
                w_sb[wn] = []
                for kc in range(NKC):
                    wt = pool.tile([128, C], F32R, name=f"{wn}_{kc}", tag=f"{wn}_{kc}")
                    nc.gpsimd.dma_start(wt[:], w_d[wn][kc * 128 : (kc + 1) * 128, :])
                    w_sb[wn].append(wt)

            load_w("Wo", wpool)
            bq_col = wpool.tile([128, NKC], F32, name="bq_col", tag="bq_col")
            bk_col = wpool.tile([128, NKC], F32, name="bk_col", tag="bk_col")
            for col, src in ((bq_col, b_d["bq"]), (bk_col, b_d["bk"])):
                for kc in range(NKC):
                    nc.gpsimd.dma_start(
                        col[:, kc : kc + 1], src[kc * 128 : (kc + 1) * 128]
                    )
            bv_row = wpool.tile([1, C], F32R, name="bv_row", tag="bv_row")
            bo_row = wpool.tile([1, C], F32R, name="bo_row", tag="bo_row")
            nc.gpsimd.dma_start(bv_row[0:1, :], b_d["bv"])
            nc.gpsimd.dma_start(bo_row[0:1, :], b_d["bo"])
            ones_col = wpool.tile([128, 1], F32R, name="ones_col", tag="ones_col")
            nc.gpsimd.memset(ones_col[:].bitcast(mybir.dt.uint32), 0x3F800000)

            # ---- transposed-load + projection phase -------------------
            # Swizzled DRAM view: l = 512 s + 128 lt + 32 b + pp,
            # c = 128 k + 32 a + ff. Staging tile [128=(a pp), 512=(lt b
            # ff)] holds 32x32 blocks placed transposed with natural
            # content, so one DVE stream-transpose yields
            # X^T[128k:+128, 512s:+512]. One DMA per (k, a) keeps the
            # DRAM access pattern within 3 dims.
            v_sb = [
                vpool.tile([128, C], F32R, name=f"v{m}", tag=f"v{m}")
                for m in range(NKT)
            ]

            with (
                tc.tile_pool(name="wproj", bufs=1) as wproj,
                tc.tile_pool(name="stg", bufs=6) as stg_pool,
                tc.tile_pool(name="xts", bufs=4) as xts_pool,
                tc.tile_pool(name="psA", bufs=6, space="PSUM") as ps_pool,
            ):
                load_w("Wv", wproj)
                load_w("Wk", wproj)
                load_w("Wq", wproj)

                def swizzled(dram):
                    return dram.rearrange(
                        "(s lt b pp) (k a ff) -> s k a pp lt b ff",
                        lt=4, b=4, pp=32, k=NKC, a=4, ff=32,
                    )

                def load_xt_slab(view, s):
                    # One chain per (slab, c-chunk): 4 DMAs (one per
                    # 32-partition group) -> DVE 32x32 stream-transpose ->
                    # ACT identity (f32r rounding).
                    xt = []
                    for kc in range(NKC):
                        stg = stg_pool.tile([128, 512], F32R, name="stg", tag="stg")
                        for a in range(4):
                            eng = nc.sync if (kc * 4 + a) % 2 == 0 else nc.scalar
                            eng.dma_start(
                                stg[32 * a : 32 * a + 32, :], view[s, kc, a]
                            )
                        xtf = stg_pool.tile([128, 512], F32, name="xtf", tag="xtf")
                        nc.vector.transpose(xtf[:], stg[:].bitcast(F32))
                        xk = xts_pool.tile(
                            [128, 512], F32R, name=f"xt{kc}", tag=f"xt{kc}"
                        )
                        nc.scalar.activation(xk[:], xtf[:], AFT.Identity)
                        xt.append(xk)
                    return xt

                # K^T and Q^T in [C_out, L] layout with per-partition bias.
                def project_T(dram, wn, b_col, Lx, pool, basename):
                    res = [
                        pool.tile(
                            [128, Lx], F32R, name=f"{basename}{j}", tag=f"{basename}{j}"
                        )
                        for j in range(NKC)
                    ]
                    view = swizzled(dram)
                    for s in range(Lx // 512):
                        xt = load_xt_slab(view, s)
                        for j in range(NKC):
                            ps = ps_pool.tile([128, 512], F32, name="ps", tag="ps")
                            for kc in range(NKC):
                                nc.tensor.matmul(
                                    ps[:],
                                    (w_sb[wn][kc][:, j * 128 : (j + 1) * 128]),
                                    (xt[kc][:]),
                                    start=(kc == 0),
                                    stop=(kc == NKC - 1),
                                )
                            nc.scalar.activation(
                                res[j][:, s * 512 : (s + 1) * 512],
                                ps[:],
                                AFT.Identity,
                                bias=b_col[:, j : j + 1],
                            )
                    return res

                kt = project_T(k_d, "Wk", bk_col, LK, kt_pool, "kt")
                qt = project_T(q_d, "Wq", bq_col, LQ, qt_pool, "qt")

                # V in natural [LK, C] layout: V = Xv @ Wv (bias folded
                # into O^T later via bv x d).
                vview = swizzled(v_d)
                for s in range(LK // 512):
                    xt = load_xt_slab(vview, s)
                    for mt in range(4):
                        m = 4 * s + mt
                        ps = ps_pool.tile([128, 512], F32, name="ps", tag="ps")
                        for kc in range(NKC):
                            nc.tensor.matmul(
                                ps[:],
                                (xt[kc][:, mt * 128 : (mt + 1) * 128]),
                                (w_sb["Wv"][kc][:]),
                                start=(kc == 0),
                                stop=(kc == NKC - 1),
                            )
                        nc.vector.tensor_copy(v_sb[m][:], ps[:])

            # ---- attention phase --------------------------------------
            with (
                tc.tile_pool(name="ptp", bufs=1) as pt_pool,
                tc.tile_pool(name="otp", bufs=1) as ot_pool,
                tc.tile_pool(name="ypool", bufs=2) as y_pool,
                tc.tile_pool(name="dpool", bufs=1) as d_pool,
                tc.tile_pool(name="psS", bufs=3, space="PSUM") as ps_s,
                tc.tile_pool(name="psD", bufs=1, space="PSUM") as ps_d,
                tc.tile_pool(name="psO", bufs=2, space="PSUM") as ps_o,
                tc.tile_pool(name="psY", bufs=2, space="PSUM") as ps_y,
            ):
                for blk in range(NQB):
                    qs = slice(blk * 512, (blk + 1) * 512)
                    # P^T = exp(s * S^T) tile-by-tile over LK
                    pt = [
                        pt_pool.tile([128, 512], F32R, name=f"pt{lk}", tag=f"pt{lk}")
                        for lk in range(NKT)
                    ]
                    for lk in range(NKT):
                        ps = ps_s.tile([128, 512], F32, name="ps_s", tag="s")
                        for kc in range(NKC):
                            nc.tensor.matmul(
                                ps[:],
                                (kt[kc][:, lk * 128 : (lk + 1) * 128]),
                                (qt[kc][:, qs]),
                                start=(kc == 0),
                                stop=(kc == NKC - 1),
                            )
                        nc.scalar.activation(
                            pt[lk][:], ps[:], AFT.Exp, scale=SM_SCALE
                        )

                    # d = 1^T P^T : [1, 512] row of softmax denominators
                    psd = ps_d.tile([1, 512], F32, name="psd", tag="d")
                    for lk in range(NKT):
                        nc.tensor.matmul(
                            psd[:],
                            (ones_col[:]),
                            (pt[lk][:]),
                            start=(lk == 0),
                            stop=(lk == NKT - 1),
                        )
                    d_row = d_pool.tile([1, 512], F32R, name=f"d_row{blk}", tag=f"d_row{blk}")
                    nc.vector.tensor_copy(d_row[:], psd[:])
                    # scatter the row to column layout, then reciprocal
                    d_col = d_pool.tile([128, 4], F32R, name=f"d_col{blk}", tag=f"d_col{blk}")
                    for mm in range(4):
                        nc.sync.dma_start(
                            d_col[:, mm : mm + 1], d_row[0:1, mm * 128 : (mm + 1) * 128]
                        )
                    d_rcp = d_pool.tile([128, 4], F32, name=f"d_rcp{blk}", tag=f"d_rcp{blk}")
                    nc.vector.reciprocal(d_rcp[:], d_col[:].bitcast(F32))

                    # O^T = V^T P^T + bv x d
                    ot = [
                        ot_pool.tile([128, 512], F32R, name=f"ot{cc}", tag=f"ot{cc}")
                        for cc in range(NKC)
                    ]
                    for cc in range(NKC):
                        pso = ps_o.tile([128, 512], F32, name="ps_o", tag="o")
                        for lk in range(NKT):
                            nc.tensor.matmul(
                                pso[:],
                                (v_sb[lk][:, cc * 128 : (cc + 1) * 128]),
                                (pt[lk][:]),
                                start=(lk == 0),
                                stop=False,
                            )
                        nc.tensor.matmul(
                            pso[:],
                            (bv_row[0:1, cc * 128 : (cc + 1) * 128]),
                            (d_row[:]),
                            start=False,
                            stop=True,
                        )
                        nc.vector.tensor_copy(ot[cc][:], pso[:])

                    # Y = (O^T)^T Wo + d x bo, then row-scale by 1/d
                    for mm in range(4):
                        psy = ps_y.tile([128, 512], F32, name="ps_y", tag="y")
                        for cc in range(NKC):
                            nc.tensor.matmul(
                                psy[:],
                                (ot[cc][:, mm * 128 : (mm + 1) * 128]),
                                (w_sb["Wo"][cc][:]),
                                start=(cc == 0),
                                stop=False,
                            )
                        nc.tensor.matmul(
                            psy[:],
                            (d_row[0:1, mm * 128 : (mm + 1) * 128]),
                            (bo_row[:]),
                            start=False,
                            stop=True,
                        )
                        y_sb = y_pool.tile([128, C], F32, name="y_sb", tag="y_sb")
                        nc.scalar.activation(
                            y_sb[:], psy[:], AFT.Copy, scale=d_rcp[:, mm : mm + 1]
                        )
                        row0 = blk * 512 + mm * 128
                        nc.sync.dma_start(out_d[row0 : row0 + 128, :], y_sb[:])

    nc.compile()
    return nc


_NC_CACHE = None


def get_nc():
    global _NC_CACHE
    if _NC_CACHE is None:
        _NC_CACHE = build_nc()
    return _NC_CACHE


def make_in_maps(inputs):
    """Shard the full inputs into 8 per-core input maps."""
    full_q = np.asarray(inputs["query"], dtype=np.float32)
    full_k = np.asarray(inputs["key"], dtype=np.float32)
    full_v = np.asarray(inputs["value"], dtype=np.float32)
    shared = {
        n: np.ascontiguousarray(np.asarray(inputs[n], dtype=np.float32))
        for n in ("Wq", "bq", "Wk", "bk", "Wv", "bv", "Wo", "bo")
    }
    in_maps = []
    for c in range(N_CORES):
        b, h = divmod(c, 2)
        m = {
            "query": np.ascontiguousarray(full_q[b, h * LQ : (h + 1) * LQ]),
            "key": np.ascontiguousarray(full_k[b]),
            "value": np.ascontiguousarray(full_v[b]),
        }
        m.update(shared)
        in_maps.append(m)
    return in_maps


def kernel(**inputs) -> np.ndarray:
    from concourse.bass_utils import run_bass_kernel_spmd

    nc = get_nc()
    in_maps = make_in_maps(inputs)
    res = run_bass_kernel_spmd(nc, in_maps, list(range(N_CORES)))
    out = np.empty((B, L, C), dtype=np.float32)
    for c in range(N_CORES):
        b, h = divmod(c, 2)
        out[b, h * LQ : (h + 1) * LQ] = res.results[c]["out"]
    return out


# revision 12
# speedup vs baseline: 1.1250x; 1.0544x over previous
"""Single-head attention (B=4, L=2048, C=512) on 8 NeuronCores.

Sharding: data-parallel over batch (4) x query-halves (2) = 8 shards.
Each core gets a [1024, 512] query slice plus the full [2048, 512] K/V
for its batch, and computes the complete attention output for its rows.
No cross-core communication is needed.

Per-core dataflow (all matmuls on the PE array in fp32r):
  - inputs are DMA'd with a 32x32 block-swizzled access pattern and
    fixed up with the DVE 32x32 stream-transpose, giving X^T in SBUF
    ([C-chunk on partitions, L free]) without using PE transposes.
  - Q^T = Wq^T X^T + bq (bias via per-partition activation), same K^T.
  - V   = (X_v^T)^T Wv in natural [L, C] layout (bias folded later).
  - S^T = K Q^T  -> P^T = exp(s * S^T)   (no row-max subtraction:
    scores are ~N(0,1), |s*S| < ~7, exp is fp32-safe).
  - d   = 1^T P^T (row sums as a [1, Lq] row via K=128 matmul with ones)
  - O^T = V^T P^T + bv x d   (bv outer product folds the V bias)
  - Y   = (O^T)^T Wo + d x bo, then scaled by 1/d per row on PSUM->SBUF
    copy (reciprocal row scattered to column layout by a tiny DMA).
"""

import sys
import numpy as np

for _p in ("/opt/trn_rl_repo",):
    if _p not in sys.path:
        sys.path.append(_p)

import concourse.bass as bass  # noqa: E402
import concourse.tile as tile  # noqa: E402
from concourse import bacc, mybir  # noqa: E402

F32 = mybir.dt.float32
F32R = mybir.dt.float32r
BF16 = mybir.dt.bfloat16
AFT = mybir.ActivationFunctionType

B, L, C = 4, 2048, 512
N_CORES = 8
LQ = (B * L) // N_CORES  # 1024 query rows per core
LK = L  # 2048 key rows per core
SM_SCALE = 1.0 / float(np.sqrt(C))

NKC = C // 128  # 4 contraction chunks
NQB = LQ // 512  # 2 query blocks
NKT = LK // 128  # 16 key tiles


def build_nc():
    """Build the per-core Bass module (identical SPMD program on 8 cores)."""
    nc = bacc.Bacc("TRN2", target_bir_lowering=False, debug=False)

    # hi/lo bf16 halves stacked vertically: rows [0,L) = bf16(x),
    # rows [L,2L) = bf16(x - hi). One XBAR transpose per c-chunk yields
    # [128, 2L] = [hi^T | lo^T].
    qhl_d = nc.dram_tensor("query_hl", [2 * LQ, C], BF16, kind="ExternalInput").ap()
    khl_d = nc.dram_tensor("key_hl", [2 * LK, C], BF16, kind="ExternalInput").ap()
    vhl_d = nc.dram_tensor("value_hl", [2 * LK, C], BF16, kind="ExternalInput").ap()
    w_d = {
        w: nc.dram_tensor(w, [C, C], F32R, kind="ExternalInput").ap()
        for w in ("Wq", "Wk", "Wv", "Wo")
    }
    b_d = {
        b: nc.dram_tensor(b, [C], F32R if b in ("bv", "bo") else F32,
                          kind="ExternalInput").ap()
        for b in ("bq", "bk", "bv", "bo")
    }
    out_d = nc.dram_tensor("out", [LQ, C], F32, kind="ExternalOutput").ap()

    with tile.TileContext(nc) as tc:
        with (
            tc.tile_pool(name="wpool", bufs=1) as wpool,
            tc.tile_pool(name="vpool", bufs=1) as vpool,
            tc.tile_pool(name="ktp", bufs=1) as kt_pool,
            tc.tile_pool(name="qtp", bufs=1) as qt_pool,
        ):
            # ---- long-lived small constants ---------------------------
            w_sb = {}

            def load_w(wn, pool):
                w_sb[wn] = []
                for kc in range(NKC):
                    wt = pool.tile([128, C], F32R, name=f"{wn}_{kc}", tag=f"{wn}_{kc}")
                    nc.gpsimd.dma_start(wt[:], w_d[wn][kc * 128 : (kc + 1) * 128, :])
                    w_sb[wn].append(wt)

            load_w("Wo", wpool)
            bq_col = wpool.tile([128, NKC], F32, name="bq_col", tag="bq_col")
            bk_col = wpool.tile([128, NKC], F32, name="bk_col", tag="bk_col")
            for col, src in ((bq_col, b_d["bq"]), (bk_col, b_d["bk"])):
                for kc in range(NKC):
                    nc.gpsimd.dma_start(
                        col[:, kc : kc + 1], src[kc * 128 : (kc + 1) * 128]
                    )
            bv_row = wpool.tile([1, C], F32R, name="bv_row", tag="bv_row")
            bo_row = wpool.tile([1, C], F32R, name="bo_row", tag="bo_row")
            nc.gpsimd.dma_start(bv_row[0:1, :], b_d["bv"])
            nc.gpsimd.dma_start(bo_row[0:1, :], b_d["bo"])
            ones_col = wpool.tile([128, 1], F32R, name="ones_col", tag="ones_col")
            nc.gpsimd.memset(ones_col[:].bitcast(mybir.dt.uint32), 0x3F800000)

            # ---- transposed-load + projection phase -------------------
            # X^T via the HW XBAR transpose: the host splits each fp32
            # input into bf16 hi + bf16 lo (residual). Both halves are
            # DMA'd with transpose=True into [128, L] bf16 tiles per
            # c-chunk, and one DVE add reconstructs X^T in f32r to
            # ~2^-16 relative accuracy.
            v_sb = [
                vpool.tile([128, C], F32R, name=f"v{m}", tag=f"v{m}")
                for m in range(NKT)
            ]

            with (
                tc.tile_pool(name="wproj", bufs=1) as wproj,
                tc.tile_pool(name="hilo", bufs=3) as hilo_pool,
                tc.tile_pool(name="xts", bufs=4) as xts_pool,
                tc.tile_pool(name="psA", bufs=6, space="PSUM") as ps_pool,
            ):
                load_w("Wv", wproj)
                load_w("Wk", wproj)
                load_w("Wq", wproj)

                def load_xt_chunk(hl_d, kc, Lx):
                    # One XBAR transpose per c-chunk gives [hi^T | lo^T];
                    # per-slab DVE adds produce f32r X^T slab tiles.
                    # All transposes stay on the sync ring: concurrent
                    # XBAR transposes on both HWDGE rings corrupt data.
                    hl = hilo_pool.tile([128, 2 * Lx], BF16, name="hl", tag="hl")
                    nc.sync.dma_start(
                        hl[:], hl_d[:, kc * 128 : (kc + 1) * 128], transpose=True
                    )
                    slabs = []
                    for s in range(Lx // 512):
                        xk = xts_pool.tile(
                            [128, 512], F32R, name=f"xt{kc}", tag=f"xt{kc}"
                        )
                        nc.vector.tensor_add(
                            xk[:],
                            hl[:, s * 512 : (s + 1) * 512],
                            hl[:, Lx + s * 512 : Lx + (s + 1) * 512],
                        )
                        slabs.append(xk)
                    return slabs

                def load_xt(hl_d, Lx):
                    # xt[kc][s] = X^T[128kc:+128, 512s:+512] as f32r
                    return [load_xt_chunk(hl_d, kc, Lx) for kc in range(NKC)]

                # K^T and Q^T in [C_out, L] layout with per-partition bias.
                def project_T(hl_d, wn, b_col, Lx, pool, basename):
                    res = [
                        pool.tile(
                            [128, Lx], F32R, name=f"{basename}{j}", tag=f"{basename}{j}"
                        )
                        for j in range(NKC)
                    ]
                    xt = load_xt(hl_d, Lx)
                    for s in range(Lx // 512):
                        for j in range(NKC):
                            ps = ps_pool.tile([128, 512], F32, name="ps", tag="ps")
                            for kc in range(NKC):
                                nc.tensor.matmul(
                                    ps[:],
                                    (w_sb[wn][kc][:, j * 128 : (j + 1) * 128]),
                                    (xt[kc][s][:]),
                                    start=(kc == 0),
                                    stop=(kc == NKC - 1),
                                )
                            nc.scalar.activation(
                                res[j][:, s * 512 : (s + 1) * 512],
                                ps[:],
                                AFT.Identity,
                                bias=b_col[:, j : j + 1],
                            )
                    return res

                kt = project_T(khl_d, "Wk", bk_col, LK, kt_pool, "kt")
                qt = project_T(qhl_d, "Wq", bq_col, LQ, qt_pool, "qt")

                # V in natural [LK, C] layout: V = Xv @ Wv (bias folded
                # into O^T later via bv x d).
                xtv = load_xt(vhl_d, LK)
                for s in range(LK // 512):
                    for mt in range(4):
                        m = 4 * s + mt
                        ps = ps_pool.tile([128, 512], F32, name="ps", tag="ps")
                        for kc in range(NKC):
                            nc.tensor.matmul(
                                ps[:],
                                (xtv[kc][s][:, mt * 128 : (mt + 1) * 128]),
                                (w_sb["Wv"][kc][:]),
                                start=(kc == 0),
                                stop=(kc == NKC - 1),
                            )
                        nc.vector.tensor_copy(v_sb[m][:], ps[:])

            # ---- attention phase --------------------------------------
            with (
                tc.tile_pool(name="ptp", bufs=1) as pt_pool,
                tc.tile_pool(name="otp", bufs=1) as ot_pool,
                tc.tile_pool(name="ypool", bufs=2) as y_pool,
                tc.tile_pool(name="dpool", bufs=1) as d_pool,
                tc.tile_pool(name="psS", bufs=3, space="PSUM") as ps_s,
                tc.tile_pool(name="psD", bufs=1, space="PSUM") as ps_d,
                tc.tile_pool(name="psO", bufs=2, space="PSUM") as ps_o,
                tc.tile_pool(name="psY", bufs=2, space="PSUM") as ps_y,
            ):
                for blk in range(NQB):
                    qs = slice(blk * 512, (blk + 1) * 512)
                    # P^T = exp(s * S^T) tile-by-tile over LK
                    pt = [
                        pt_pool.tile([128, 512], F32R, name=f"pt{lk}", tag=f"pt{lk}")
                        for lk in range(NKT)
                    ]
                    for lk in range(NKT):
                        ps = ps_s.tile([128, 512], F32, name="ps_s", tag="s")
                        for kc in range(NKC):
                            nc.tensor.matmul(
                                ps[:],
                                (kt[kc][:, lk * 128 : (lk + 1) * 128]),
                                (qt[kc][:, qs]),
                                start=(kc == 0),
                                stop=(kc == NKC - 1),
                            )
                        nc.scalar.activation(
                            pt[lk][:], ps[:], AFT.Exp, scale=SM_SCALE
                        )

                    # d = 1^T P^T : [1, 512] row of softmax denominators
                    psd = ps_d.tile([1, 512], F32, name="psd", tag="d")
                    for lk in range(NKT):
                        nc.tensor.matmul(
                            psd[:],
                            (ones_col[:]),
                            (pt[lk][:]),
                            start=(lk == 0),
                            stop=(lk == NKT - 1),
                        )
                    d_row = d_pool.tile([1, 512], F32R, name=f"d_row{blk}", tag=f"d_row{blk}")
                    nc.vector.tensor_copy(d_row[:], psd[:])
                    # scatter the row to column layout, then reciprocal
                    d_col = d_pool.tile([128, 4], F32R, name=f"d_col{blk}", tag=f"d_col{blk}")
                    for mm in range(4):
                        nc.gpsimd.dma_start(
                            d_col[:, mm : mm + 1], d_row[0:1, mm * 128 : (mm + 1) * 128]
                        )
                    d_rcp = d_pool.tile([128, 4], F32, name=f"d_rcp{blk}", tag=f"d_rcp{blk}")
                    nc.vector.reciprocal(d_rcp[:], d_col[:].bitcast(F32))

                    # O^T = V^T P^T + bv x d
                    ot = [
                        ot_pool.tile([128, 512], F32R, name=f"ot{cc}", tag=f"ot{cc}")
                        for cc in range(NKC)
                    ]
                    for cc in range(NKC):
                        pso = ps_o.tile([128, 512], F32, name="ps_o", tag="o")
                        for lk in range(NKT):
                            nc.tensor.matmul(
                                pso[:],
                                (v_sb[lk][:, cc * 128 : (cc + 1) * 128]),
                                (pt[lk][:]),
                                start=(lk == 0),
                                stop=False,
                            )
                        nc.tensor.matmul(
                            pso[:],
                            (bv_row[0:1, cc * 128 : (cc + 1) * 128]),
                            (d_row[:]),
                            start=False,
                            stop=True,
                        )
                        nc.vector.tensor_copy(ot[cc][:], pso[:])

                    # Y = (O^T)^T Wo + d x bo, then row-scale by 1/d
                    for mm in range(4):
                        psy = ps_y.tile([128, 512], F32, name="ps_y", tag="y")
                        for cc in range(NKC):
                            nc.tensor.matmul(
                                psy[:],
                                (ot[cc][:, mm * 128 : (mm + 1) * 128]),
                                (w_sb["Wo"][cc][:]),
                                start=(cc == 0),
                                stop=False,
                            )
                        nc.tensor.matmul(
                            psy[:],
                            (d_row[0:1, mm * 128 : (mm + 1) * 128]),
                            (bo_row[:]),
                            start=False,
                            stop=True,
                        )
                        y_sb = y_pool.tile([128, C], F32, name="y_sb", tag="y_sb")
                        nc.scalar.activation(
                            y_sb[:], psy[:], AFT.Copy, scale=d_rcp[:, mm : mm + 1]
                        )
                        row0 = blk * 512 + mm * 128
                        nc.scalar.dma_start(out_d[row0 : row0 + 128, :], y_sb[:])

    nc.compile()
    return nc


_NC_CACHE = None


def get_nc():
    global _NC_CACHE
    if _NC_CACHE is None:
        _NC_CACHE = build_nc()
    return _NC_CACHE


def _hilo(x):
    import ml_dtypes

    hi = x.astype(ml_dtypes.bfloat16)
    lo = (x - hi.astype(np.float32)).astype(ml_dtypes.bfloat16)
    return np.ascontiguousarray(np.concatenate([hi, lo], axis=0))


def make_in_maps(inputs):
    """Shard the full inputs into 8 per-core input maps (bf16 hi+lo)."""
    full_q = np.asarray(inputs["query"], dtype=np.float32)
    full_k = np.asarray(inputs["key"], dtype=np.float32)
    full_v = np.asarray(inputs["value"], dtype=np.float32)
    shared = {
        n: np.ascontiguousarray(np.asarray(inputs[n], dtype=np.float32))
        for n in ("Wq", "bq", "Wk", "bk", "Wv", "bv", "Wo", "bo")
    }
    kv_hilo = [(_hilo(full_k[b]), _hilo(full_v[b])) for b in range(B)]
    in_maps = []
    for c in range(N_CORES):
        b, h = divmod(c, 2)
        m = {
            "query_hl": _hilo(full_q[b, h * LQ : (h + 1) * LQ]),
            "key_hl": kv_hilo[b][0],
            "value_hl": kv_hilo[b][1],
        }
        m.update(shared)
        in_maps.append(m)
    return in_maps


def kernel(**inputs) -> np.ndarray:
    from concourse.bass_utils import run_bass_kernel_spmd

    nc = get_nc()
    in_maps = make_in_maps(inputs)
    res = run_bass_kernel_spmd(nc, in_maps, list(range(N_CORES)))
    out = np.empty((B, L, C), dtype=np.float32)
    for c in range(N_CORES):
        b, h = divmod(c, 2)
        out[b, h * LQ : (h + 1) * LQ] = res.results[c]["out"]
    return out


# revision 13
# speedup vs baseline: 1.1449x; 1.0177x over previous
"""Single-head attention (B=4, L=2048, C=512) on 8 NeuronCores.

Sharding: data-parallel over batch (4) x query-halves (2) = 8 shards.
Each core gets a [1024, 512] query slice plus the full [2048, 512] K/V
for its batch, and computes the complete attention output for its rows.
No cross-core communication is needed.

Per-core dataflow (all matmuls on the PE array in fp32r):
  - inputs are DMA'd with a 32x32 block-swizzled access pattern and
    fixed up with the DVE 32x32 stream-transpose, giving X^T in SBUF
    ([C-chunk on partitions, L free]) without using PE transposes.
  - Q^T = Wq^T X^T + bq (bias via per-partition activation), same K^T.
  - V   = (X_v^T)^T Wv in natural [L, C] layout (bias folded later).
  - S^T = K Q^T  -> P^T = exp(s * S^T)   (no row-max subtraction:
    scores are ~N(0,1), |s*S| < ~7, exp is fp32-safe).
  - d   = 1^T P^T (row sums as a [1, Lq] row via K=128 matmul with ones)
  - O^T = V^T P^T + bv x d   (bv outer product folds the V bias)
  - Y   = (O^T)^T Wo + d x bo, then scaled by 1/d per row on PSUM->SBUF
    copy (reciprocal row scattered to column layout by a tiny DMA).
"""

import sys
import numpy as np

for _p in ("/opt/trn_rl_repo",):
    if _p not in sys.path:
        sys.path.append(_p)

import concourse.bass as bass  # noqa: E402
import concourse.tile as tile  # noqa: E402
from concourse import bacc, mybir  # noqa: E402

F32 = mybir.dt.float32
F32R = mybir.dt.float32r
BF16 = mybir.dt.bfloat16
AFT = mybir.ActivationFunctionType

B, L, C = 4, 2048, 512
N_CORES = 8
LQ = (B * L) // N_CORES  # 1024 query rows per core
LK = L  # 2048 key rows per core
SM_SCALE = 1.0 / float(np.sqrt(C))

NKC = C // 128  # 4 contraction chunks
NQB = LQ // 512  # 2 query blocks
NKT = LK // 128  # 16 key tiles


def build_nc():
    """Build the per-core Bass module (identical SPMD program on 8 cores)."""
    nc = bacc.Bacc("TRN2", target_bir_lowering=False, debug=False)

    # hi/lo bf16 halves stacked vertically: rows [0,L) = bf16(x),
    # rows [L,2L) = bf16(x - hi). One XBAR transpose per c-chunk yields
    # [128, 2L] = [hi^T | lo^T].
    qhl_d = nc.dram_tensor("query_hl", [2 * LQ, C], BF16, kind="ExternalInput").ap()
    khl_d = nc.dram_tensor("key_hl", [2 * LK, C], BF16, kind="ExternalInput").ap()
    vhl_d = nc.dram_tensor("value_hl", [2 * LK, C], BF16, kind="ExternalInput").ap()
    w_d = {
        w: nc.dram_tensor(w, [C, C], F32R, kind="ExternalInput").ap()
        for w in ("Wq", "Wk", "Wv", "Wo")
    }
    b_d = {
        b: nc.dram_tensor(b, [C], F32R if b in ("bv", "bo") else F32,
                          kind="ExternalInput").ap()
        for b in ("bq", "bk", "bv", "bo")
    }
    out_d = nc.dram_tensor("out", [LQ, C], F32, kind="ExternalOutput").ap()

    with tile.TileContext(nc) as tc:
        with (
            tc.tile_pool(name="wpool", bufs=1) as wpool,
            tc.tile_pool(name="vpool", bufs=1) as vpool,
            tc.tile_pool(name="ktp", bufs=1) as kt_pool,
            tc.tile_pool(name="qtp", bufs=1) as qt_pool,
        ):
            # ---- long-lived small constants ---------------------------
            w_sb = {}

            def load_w(wn, pool):
                w_sb[wn] = []
                for kc in range(NKC):
                    wt = pool.tile([128, C], F32R, name=f"{wn}_{kc}", tag=f"{wn}_{kc}")
                    nc.scalar.dma_start(wt[:], w_d[wn][kc * 128 : (kc + 1) * 128, :])
                    w_sb[wn].append(wt)

            load_w("Wo", wpool)
            bq_col = wpool.tile([128, NKC], F32, name="bq_col", tag="bq_col")
            bk_col = wpool.tile([128, NKC], F32, name="bk_col", tag="bk_col")
            for col, src in ((bq_col, b_d["bq"]), (bk_col, b_d["bk"])):
                for kc in range(NKC):
                    nc.scalar.dma_start(
                        col[:, kc : kc + 1], src[kc * 128 : (kc + 1) * 128]
                    )
            bv_row = wpool.tile([1, C], F32R, name="bv_row", tag="bv_row")
            bo_row = wpool.tile([1, C], F32R, name="bo_row", tag="bo_row")
            nc.scalar.dma_start(bv_row[0:1, :], b_d["bv"])
            nc.scalar.dma_start(bo_row[0:1, :], b_d["bo"])
            ones_col = wpool.tile([128, 1], F32R, name="ones_col", tag="ones_col")
            nc.gpsimd.memset(ones_col[:].bitcast(mybir.dt.uint32), 0x3F800000)

            # ---- transposed-load + projection phase -------------------
            # X^T via the HW XBAR transpose: the host splits each fp32
            # input into bf16 hi + bf16 lo (residual). Both halves are
            # DMA'd with transpose=True into [128, L] bf16 tiles per
            # c-chunk, and one DVE add reconstructs X^T in f32r to
            # ~2^-16 relative accuracy.
            v_sb = [
                vpool.tile([128, C], F32R, name=f"v{m}", tag=f"v{m}")
                for m in range(NKT)
            ]

            with (
                tc.tile_pool(name="wproj", bufs=1) as wproj,
                tc.tile_pool(name="hilo", bufs=3) as hilo_pool,
                tc.tile_pool(name="xts", bufs=4) as xts_pool,
                tc.tile_pool(name="psA", bufs=6, space="PSUM") as ps_pool,
            ):
                load_w("Wv", wproj)
                load_w("Wk", wproj)
                load_w("Wq", wproj)

                def load_xt_chunk(hl_d, kc, Lx):
                    # One XBAR transpose per c-chunk gives [hi^T | lo^T];
                    # per-slab DVE adds produce f32r X^T slab tiles.
                    # All transposes stay on the sync ring: concurrent
                    # XBAR transposes on both HWDGE rings corrupt data.
                    hl = hilo_pool.tile([128, 2 * Lx], BF16, name="hl", tag="hl")
                    nc.sync.dma_start(
                        hl[:], hl_d[:, kc * 128 : (kc + 1) * 128], transpose=True
                    )
                    slabs = []
                    for s in range(Lx // 512):
                        xk = xts_pool.tile(
                            [128, 512], F32R, name=f"xt{kc}", tag=f"xt{kc}"
                        )
                        nc.vector.tensor_add(
                            xk[:],
                            hl[:, s * 512 : (s + 1) * 512],
                            hl[:, Lx + s * 512 : Lx + (s + 1) * 512],
                        )
                        slabs.append(xk)
                    return slabs

                def load_xt(hl_d, Lx):
                    # xt[kc][s] = X^T[128kc:+128, 512s:+512] as f32r
                    return [load_xt_chunk(hl_d, kc, Lx) for kc in range(NKC)]

                # K^T and Q^T in [C_out, L] layout with per-partition bias.
                def project_T(hl_d, wn, b_col, Lx, pool, basename):
                    res = [
                        pool.tile(
                            [128, Lx], F32R, name=f"{basename}{j}", tag=f"{basename}{j}"
                        )
                        for j in range(NKC)
                    ]
                    xt = load_xt(hl_d, Lx)
                    for s in range(Lx // 512):
                        for j in range(NKC):
                            ps = ps_pool.tile([128, 512], F32, name="ps", tag="ps")
                            for kc in range(NKC):
                                nc.tensor.matmul(
                                    ps[:],
                                    (w_sb[wn][kc][:, j * 128 : (j + 1) * 128]),
                                    (xt[kc][s][:]),
                                    start=(kc == 0),
                                    stop=(kc == NKC - 1),
                                )
                            nc.scalar.activation(
                                res[j][:, s * 512 : (s + 1) * 512],
                                ps[:],
                                AFT.Identity,
                                bias=b_col[:, j : j + 1],
                            )
                    return res

                kt = project_T(khl_d, "Wk", bk_col, LK, kt_pool, "kt")
                qt = project_T(qhl_d, "Wq", bq_col, LQ, qt_pool, "qt")

                # V in natural [LK, C] layout: V = Xv @ Wv (bias folded
                # into O^T later via bv x d).
                xtv = load_xt(vhl_d, LK)
                for s in range(LK // 512):
                    for mt in range(4):
                        m = 4 * s + mt
                        ps = ps_pool.tile([128, 512], F32, name="ps", tag="ps")
                        for kc in range(NKC):
                            nc.tensor.matmul(
                                ps[:],
                                (xtv[kc][s][:, mt * 128 : (mt + 1) * 128]),
                                (w_sb["Wv"][kc][:]),
                                start=(kc == 0),
                                stop=(kc == NKC - 1),
                            )
                        nc.vector.tensor_copy(v_sb[m][:], ps[:])

            # ---- attention phase --------------------------------------
            with (
                tc.tile_pool(name="ptp", bufs=1) as pt_pool,
                tc.tile_pool(name="otp", bufs=1) as ot_pool,
                tc.tile_pool(name="ypool", bufs=2) as y_pool,
                tc.tile_pool(name="dpool", bufs=1) as d_pool,
                tc.tile_pool(name="psS", bufs=3, space="PSUM") as ps_s,
                tc.tile_pool(name="psD", bufs=1, space="PSUM") as ps_d,
                tc.tile_pool(name="psO", bufs=2, space="PSUM") as ps_o,
                tc.tile_pool(name="psY", bufs=2, space="PSUM") as ps_y,
            ):
                for blk in range(NQB):
                    qs = slice(blk * 512, (blk + 1) * 512)
                    # P^T = exp(s * S^T) tile-by-tile over LK
                    pt = [
                        pt_pool.tile([128, 512], F32R, name=f"pt{lk}", tag=f"pt{lk}")
                        for lk in range(NKT)
                    ]
                    for lk in range(NKT):
                        ps = ps_s.tile([128, 512], F32, name="ps_s", tag="s")
                        for kc in range(NKC):
                            nc.tensor.matmul(
                                ps[:],
                                (kt[kc][:, lk * 128 : (lk + 1) * 128]),
                                (qt[kc][:, qs]),
                                start=(kc == 0),
                                stop=(kc == NKC - 1),
                            )
                        nc.scalar.activation(
                            pt[lk][:], ps[:], AFT.Exp, scale=SM_SCALE
                        )

                    # d = 1^T P^T : [1, 512] row of softmax denominators
                    psd = ps_d.tile([1, 512], F32, name="psd", tag="d")
                    for lk in range(NKT):
                        nc.tensor.matmul(
                            psd[:],
                            (ones_col[:]),
                            (pt[lk][:]),
                            start=(lk == 0),
                            stop=(lk == NKT - 1),
                        )
                    d_row = d_pool.tile([1, 512], F32R, name=f"d_row{blk}", tag=f"d_row{blk}")
                    nc.vector.tensor_copy(d_row[:], psd[:])
                    # scatter the row to column layout, then reciprocal
                    d_col = d_pool.tile([128, 4], F32R, name=f"d_col{blk}", tag=f"d_col{blk}")
                    for mm in range(4):
                        nc.gpsimd.dma_start(
                            d_col[:, mm : mm + 1], d_row[0:1, mm * 128 : (mm + 1) * 128]
                        )
                    d_rcp = d_pool.tile([128, 4], F32, name=f"d_rcp{blk}", tag=f"d_rcp{blk}")
                    nc.vector.reciprocal(d_rcp[:], d_col[:].bitcast(F32))

                    # O^T = V^T P^T + bv x d
                    ot = [
                        ot_pool.tile([128, 512], F32R, name=f"ot{cc}", tag=f"ot{cc}")
                        for cc in range(NKC)
                    ]
                    for cc in range(NKC):
                        pso = ps_o.tile([128, 512], F32, name="ps_o", tag="o")
                        for lk in range(NKT):
                            nc.tensor.matmul(
                                pso[:],
                                (v_sb[lk][:, cc * 128 : (cc + 1) * 128]),
                                (pt[lk][:]),
                                start=(lk == 0),
                                stop=False,
                            )
                        nc.tensor.matmul(
                            pso[:],
                            (bv_row[0:1, cc * 128 : (cc + 1) * 128]),
                            (d_row[:]),
                            start=False,
                            stop=True,
                        )
                        nc.vector.tensor_copy(ot[cc][:], pso[:])

                    # Y = (O^T)^T Wo + d x bo, then row-scale by 1/d
                    for mm in range(4):
                        psy = ps_y.tile([128, 512], F32, name="ps_y", tag="y")
                        for cc in range(NKC):
                            nc.tensor.matmul(
                                psy[:],
                                (ot[cc][:, mm * 128 : (mm + 1) * 128]),
                                (w_sb["Wo"][cc][:]),
                                start=(cc == 0),
                                stop=False,
                            )
                        nc.tensor.matmul(
                            psy[:],
                            (d_row[0:1, mm * 128 : (mm + 1) * 128]),
                            (bo_row[:]),
                            start=False,
                            stop=True,
                        )
                        y_sb = y_pool.tile([128, C], F32, name="y_sb", tag="y_sb")
                        nc.scalar.activation(
                            y_sb[:], psy[:], AFT.Copy, scale=d_rcp[:, mm : mm + 1]
                        )
                        row0 = blk * 512 + mm * 128
                        nc.scalar.dma_start(out_d[row0 : row0 + 128, :], y_sb[:])

    nc.compile()
    return nc


_NC_CACHE = None


def get_nc():
    global _NC_CACHE
    if _NC_CACHE is None:
        _NC_CACHE = build_nc()
    return _NC_CACHE


def _hilo(x):
    import ml_dtypes

    hi = x.astype(ml_dtypes.bfloat16)
    lo = (x - hi.astype(np.float32)).astype(ml_dtypes.bfloat16)
    return np.ascontiguousarray(np.concatenate([hi, lo], axis=0))


def make_in_maps(inputs):
    """Shard the full inputs into 8 per-core input maps (bf16 hi+lo)."""
    full_q = np.asarray(inputs["query"], dtype=np.float32)
    full_k = np.asarray(inputs["key"], dtype=np.float32)
    full_v = np.asarray(inputs["value"], dtype=np.float32)
    shared = {
        n: np.ascontiguousarray(np.asarray(inputs[n], dtype=np.float32))
        for n in ("Wq", "bq", "Wk", "bk", "Wv", "bv", "Wo", "bo")
    }
    kv_hilo = [(_hilo(full_k[b]), _hilo(full_v[b])) for b in range(B)]
    in_maps = []
    for c in range(N_CORES):
        b, h = divmod(c, 2)
        m = {
            "query_hl": _hilo(full_q[b, h * LQ : (h + 1) * LQ]),
            "key_hl": kv_hilo[b][0],
            "value_hl": kv_hilo[b][1],
        }
        m.update(shared)
        in_maps.append(m)
    return in_maps


def kernel(**inputs) -> np.ndarray:
    from concourse.bass_utils import run_bass_kernel_spmd

    nc = get_nc()
    in_maps = make_in_maps(inputs)
    res = run_bass_kernel_spmd(nc, in_maps, list(range(N_CORES)))
    out = np.empty((B, L, C), dtype=np.float32)
    for c in range(N_CORES):
        b, h = divmod(c, 2)
        out[b, h * LQ : (h + 1) * LQ] = res.results[c]["out"]
    return out


# revision 15
# speedup vs baseline: 1.1738x; 1.0252x over previous
"""Single-head attention (B=4, L=2048, C=512) on 8 NeuronCores.

Sharding: data-parallel over batch (4) x query-halves (2) = 8 shards.
Each core gets a [1024, 512] query slice plus the full [2048, 512] K/V
for its batch, and computes the complete attention output for its rows.
No cross-core communication is needed.

Per-core dataflow (all matmuls on the PE array in fp32r):
  - inputs are DMA'd with a 32x32 block-swizzled access pattern and
    fixed up with the DVE 32x32 stream-transpose, giving X^T in SBUF
    ([C-chunk on partitions, L free]) without using PE transposes.
  - Q^T = Wq^T X^T + bq (bias via per-partition activation), same K^T.
  - V   = (X_v^T)^T Wv in natural [L, C] layout (bias folded later).
  - S^T = K Q^T  -> P^T = exp(s * S^T)   (no row-max subtraction:
    scores are ~N(0,1), |s*S| < ~7, exp is fp32-safe).
  - d   = 1^T P^T (row sums as a [1, Lq] row via K=128 matmul with ones)
  - O^T = V^T P^T + bv x d   (bv outer product folds the V bias)
  - Y   = (O^T)^T Wo + d x bo, then scaled by 1/d per row on PSUM->SBUF
    copy (reciprocal row scattered to column layout by a tiny DMA).
"""

import sys
import numpy as np

for _p in ("/opt/trn_rl_repo",):
    if _p not in sys.path:
        sys.path.append(_p)

import concourse.bass as bass  # noqa: E402
import concourse.tile as tile  # noqa: E402
from concourse import bacc, mybir  # noqa: E402

F32 = mybir.dt.float32
F32R = mybir.dt.float32r
BF16 = mybir.dt.bfloat16
AFT = mybir.ActivationFunctionType

B, L, C = 4, 2048, 512
N_CORES = 8
LQ = (B * L) // N_CORES  # 1024 query rows per core
LK = L  # 2048 key rows per core
SM_SCALE = 1.0 / float(np.sqrt(C))

NKC = C // 128  # 4 contraction chunks
NQB = LQ // 512  # 2 query blocks
NKT = LK // 128  # 16 key tiles


def build_nc():
    """Build the per-core Bass module (identical SPMD program on 8 cores)."""
    nc = bacc.Bacc("TRN2", target_bir_lowering=False, debug=False)

    # hi/lo bf16 halves stacked vertically: rows [0,L) = bf16(x),
    # rows [L,2L) = bf16(x - hi). One XBAR transpose per c-chunk yields
    # [128, 2L] = [hi^T | lo^T].
    qhl_d = nc.dram_tensor("query_hl", [2 * LQ, C], BF16, kind="ExternalInput").ap()
    khl_d = nc.dram_tensor("key_hl", [2 * LK, C], BF16, kind="ExternalInput").ap()
    vhl_d = nc.dram_tensor("value_hl", [2 * LK, C], BF16, kind="ExternalInput").ap()
    w_d = {
        w: nc.dram_tensor(w, [C, C], F32R, kind="ExternalInput").ap()
        for w in ("Wq", "Wk", "Wv", "Wo")
    }
    b_d = {
        b: nc.dram_tensor(b, [C], F32R if b in ("bv", "bo") else F32,
                          kind="ExternalInput").ap()
        for b in ("bq", "bk", "bv", "bo")
    }
    out_d = nc.dram_tensor("out", [LQ, C], F32, kind="ExternalOutput").ap()

    with tile.TileContext(nc) as tc:
        with (
            tc.tile_pool(name="wpool", bufs=1) as wpool,
            tc.tile_pool(name="vpool", bufs=1) as vpool,
            tc.tile_pool(name="ktp", bufs=1) as kt_pool,
            tc.tile_pool(name="qtp", bufs=1) as qt_pool,
        ):
            # ---- long-lived small constants ---------------------------
            w_sb = {}

            def load_w(wn, pool):
                wt = pool.tile([128, NKC * C], F32R, name=f"{wn}_t", tag=f"{wn}_t")
                nc.scalar.dma_start(
                    wt[:], w_d[wn].rearrange("(kc p) f -> p kc f", p=128)
                )
                w_sb[wn] = [wt[:, kc * C : (kc + 1) * C] for kc in range(NKC)]

            load_w("Wo", wpool)
            bq_col = wpool.tile([128, NKC], F32, name="bq_col", tag="bq_col")
            bk_col = wpool.tile([128, NKC], F32, name="bk_col", tag="bk_col")
            for col, bsrc in ((bq_col, b_d["bq"]), (bk_col, b_d["bk"])):
                nc.scalar.dma_start(
                    col[:], bsrc.rearrange("(kc p) -> p kc", p=128)
                )
            bv_row = wpool.tile([1, C], F32R, name="bv_row", tag="bv_row")
            bo_row = wpool.tile([1, C], F32R, name="bo_row", tag="bo_row")
            nc.scalar.dma_start(bv_row[0:1, :], b_d["bv"])
            nc.scalar.dma_start(bo_row[0:1, :], b_d["bo"])
            ones_col = wpool.tile([128, 1], F32R, name="ones_col", tag="ones_col")
            nc.gpsimd.memset(ones_col[:].bitcast(mybir.dt.uint32), 0x3F800000)

            # ---- transposed-load + projection phase -------------------
            # X^T via the HW XBAR transpose: the host splits each fp32
            # input into bf16 hi + bf16 lo (residual). Both halves are
            # DMA'd with transpose=True into [128, L] bf16 tiles per
            # c-chunk, and one DVE add reconstructs X^T in f32r to
            # ~2^-16 relative accuracy.
            v_sb = [
                vpool.tile([128, C], F32R, name=f"v{m}", tag=f"v{m}")
                for m in range(NKT)
            ]

            with (
                tc.tile_pool(name="wproj", bufs=1) as wproj,
                tc.tile_pool(name="hilo", bufs=3) as hilo_pool,
                tc.tile_pool(name="xts", bufs=4) as xts_pool,
                tc.tile_pool(name="psA", bufs=6, space="PSUM") as ps_pool,
            ):
                load_w("Wk", wproj)
                load_w("Wq", wproj)
                load_w("Wv", wproj)

                def load_xt_chunk(hl_d, kc, Lx):
                    # One XBAR transpose per c-chunk gives [hi^T | lo^T];
                    # per-slab DVE adds produce f32r X^T slab tiles.
                    # All transposes stay on the sync ring: concurrent
                    # XBAR transposes on both HWDGE rings corrupt data.
                    hl = hilo_pool.tile([128, 2 * Lx], BF16, name="hl", tag="hl")
                    nc.sync.dma_start(
                        hl[:], hl_d[:, kc * 128 : (kc + 1) * 128], transpose=True
                    )
                    slabs = []
                    for s in range(Lx // 512):
                        xk = xts_pool.tile(
                            [128, 512], F32R, name=f"xt{kc}", tag=f"xt{kc}"
                        )
                        nc.vector.tensor_add(
                            xk[:],
                            hl[:, s * 512 : (s + 1) * 512],
                            hl[:, Lx + s * 512 : Lx + (s + 1) * 512],
                        )
                        slabs.append(xk)
                    return slabs

                def load_xt(hl_d, Lx):
                    # xt[kc][s] = X^T[128kc:+128, 512s:+512] as f32r
                    return [load_xt_chunk(hl_d, kc, Lx) for kc in range(NKC)]

                # K^T and Q^T in [C_out, L] layout with per-partition bias.
                def project_T(hl_d, wn, b_col, Lx, pool, basename):
                    res = [
                        pool.tile(
                            [128, Lx], F32R, name=f"{basename}{j}", tag=f"{basename}{j}"
                        )
                        for j in range(NKC)
                    ]
                    xt = load_xt(hl_d, Lx)
                    for s in range(Lx // 512):
                        for j in range(NKC):
                            ps = ps_pool.tile([128, 512], F32, name="ps", tag="ps")
                            for kc in range(NKC):
                                nc.tensor.matmul(
                                    ps[:],
                                    (w_sb[wn][kc][:, j * 128 : (j + 1) * 128]),
                                    (xt[kc][s][:]),
                                    start=(kc == 0),
                                    stop=(kc == NKC - 1),
                                )
                            nc.scalar.activation(
                                res[j][:, s * 512 : (s + 1) * 512],
                                ps[:],
                                AFT.Identity,
                                bias=b_col[:, j : j + 1],
                            )
                    return res

                kt = project_T(khl_d, "Wk", bk_col, LK, kt_pool, "kt")
                qt = project_T(qhl_d, "Wq", bq_col, LQ, qt_pool, "qt")

                # V in natural [LK, C] layout: V = Xv @ Wv (bias folded
                # into O^T later via bv x d).
                xtv = load_xt(vhl_d, LK)
                for s in range(LK // 512):
                    for mt in range(4):
                        m = 4 * s + mt
                        ps = ps_pool.tile([128, 512], F32, name="ps", tag="ps")
                        for kc in range(NKC):
                            nc.tensor.matmul(
                                ps[:],
                                (xtv[kc][s][:, mt * 128 : (mt + 1) * 128]),
                                (w_sb["Wv"][kc][:]),
                                start=(kc == 0),
                                stop=(kc == NKC - 1),
                            )
                        nc.vector.tensor_copy(v_sb[m][:], ps[:])

            # ---- attention phase --------------------------------------
            with (
                tc.tile_pool(name="ptp", bufs=1) as pt_pool,
                tc.tile_pool(name="otp", bufs=1) as ot_pool,
                tc.tile_pool(name="ypool", bufs=2) as y_pool,
                tc.tile_pool(name="dpool", bufs=1) as d_pool,
                tc.tile_pool(name="psS", bufs=3, space="PSUM") as ps_s,
                tc.tile_pool(name="psD", bufs=1, space="PSUM") as ps_d,
                tc.tile_pool(name="psO", bufs=2, space="PSUM") as ps_o,
                tc.tile_pool(name="psY", bufs=2, space="PSUM") as ps_y,
            ):
                for blk in range(NQB):
                    qs = slice(blk * 512, (blk + 1) * 512)
                    # P^T = exp(s * S^T) tile-by-tile over LK
                    pt = [
                        pt_pool.tile([128, 512], F32R, name=f"pt{lk}", tag=f"pt{lk}")
                        for lk in range(NKT)
                    ]
                    for lk in range(NKT):
                        ps = ps_s.tile([128, 512], F32, name="ps_s", tag="s")
                        for kc in range(NKC):
                            nc.tensor.matmul(
                                ps[:],
                                (kt[kc][:, lk * 128 : (lk + 1) * 128]),
                                (qt[kc][:, qs]),
                                start=(kc == 0),
                                stop=(kc == NKC - 1),
                            )
                        nc.scalar.activation(
                            pt[lk][:], ps[:], AFT.Exp, scale=SM_SCALE
                        )

                    # d = 1^T P^T : [1, 512] row of softmax denominators
                    psd = ps_d.tile([1, 512], F32, name="psd", tag="d")
                    for lk in range(NKT):
                        nc.tensor.matmul(
                            psd[:],
                            (ones_col[:]),
                            (pt[lk][:]),
                            start=(lk == 0),
                            stop=(lk == NKT - 1),
                        )
                    d_row = d_pool.tile([1, 512], F32R, name=f"d_row{blk}", tag=f"d_row{blk}")
                    nc.vector.tensor_copy(d_row[:], psd[:])
                    # scatter the row to column layout, then reciprocal
                    d_col = d_pool.tile([128, 4], F32R, name=f"d_col{blk}", tag=f"d_col{blk}")
                    for mm in range(4):
                        nc.gpsimd.dma_start(
                            d_col[:, mm : mm + 1], d_row[0:1, mm * 128 : (mm + 1) * 128]
                        )
                    d_rcp = d_pool.tile([128, 4], F32, name=f"d_rcp{blk}", tag=f"d_rcp{blk}")
                    nc.vector.reciprocal(d_rcp[:], d_col[:].bitcast(F32))

                    # O^T = V^T P^T + bv x d
                    ot = [
                        ot_pool.tile([128, 512], F32R, name=f"ot{cc}", tag=f"ot{cc}")
                        for cc in range(NKC)
                    ]
                    for cc in range(NKC):
                        pso = ps_o.tile([128, 512], F32, name="ps_o", tag="o")
                        for lk in range(NKT):
                            nc.tensor.matmul(
                                pso[:],
                                (v_sb[lk][:, cc * 128 : (cc + 1) * 128]),
                                (pt[lk][:]),
                                start=(lk == 0),
                                stop=False,
                            )
                        nc.tensor.matmul(
                            pso[:],
                            (bv_row[0:1, cc * 128 : (cc + 1) * 128]),
                            (d_row[:]),
                            start=False,
                            stop=True,
                        )
                        nc.vector.tensor_copy(ot[cc][:], pso[:])

                    # Y = (O^T)^T Wo + d x bo, then row-scale by 1/d
                    for mm in range(4):
                        psy = ps_y.tile([128, 512], F32, name="ps_y", tag="y")
                        for cc in range(NKC):
                            nc.tensor.matmul(
                                psy[:],
                                (ot[cc][:, mm * 128 : (mm + 1) * 128]),
                                (w_sb["Wo"][cc][:]),
                                start=(cc == 0),
                                stop=False,
                            )
                        nc.tensor.matmul(
                            psy[:],
                            (d_row[0:1, mm * 128 : (mm + 1) * 128]),
                            (bo_row[:]),
                            start=False,
                            stop=True,
                        )
                        y_sb = y_pool.tile([128, C], F32, name="y_sb", tag="y_sb")
                        nc.scalar.activation(
                            y_sb[:], psy[:], AFT.Copy, scale=d_rcp[:, mm : mm + 1]
                        )
                        row0 = blk * 512 + mm * 128
                        nc.scalar.dma_start(out_d[row0 : row0 + 128, :], y_sb[:])

    nc.compile()
    return nc


_NC_CACHE = None


def get_nc():
    global _NC_CACHE
    if _NC_CACHE is None:
        _NC_CACHE = build_nc()
    return _NC_CACHE


def _hilo(x):
    import ml_dtypes

    hi = x.astype(ml_dtypes.bfloat16)
    lo = (x - hi.astype(np.float32)).astype(ml_dtypes.bfloat16)
    return np.ascontiguousarray(np.concatenate([hi, lo], axis=0))


def make_in_maps(inputs):
    """Shard the full inputs into 8 per-core input maps (bf16 hi+lo)."""
    full_q = np.asarray(inputs["query"], dtype=np.float32)
    full_k = np.asarray(inputs["key"], dtype=np.float32)
    full_v = np.asarray(inputs["value"], dtype=np.float32)
    shared = {
        n: np.ascontiguousarray(np.asarray(inputs[n], dtype=np.float32))
        for n in ("Wq", "bq", "Wk", "bk", "Wv", "bv", "Wo", "bo")
    }
    kv_hilo = [(_hilo(full_k[b]), _hilo(full_v[b])) for b in range(B)]
    in_maps = []
    for c in range(N_CORES):
        b, h = divmod(c, 2)
        m = {
            "query_hl": _hilo(full_q[b, h * LQ : (h + 1) * LQ]),
            "key_hl": kv_hilo[b][0],
            "value_hl": kv_hilo[b][1],
        }
        m.update(shared)
        in_maps.append(m)
    return in_maps


def kernel(**inputs) -> np.ndarray:
    from concourse.bass_utils import run_bass_kernel_spmd

    nc = get_nc()
    in_maps = make_in_maps(inputs)
    res = run_bass_kernel_spmd(nc, in_maps, list(range(N_CORES)))
    out = np.empty((B, L, C), dtype=np.float32)
    for c in range(N_CORES):
        b, h = divmod(c, 2)
        out[b, h * LQ : (h + 1) * LQ] = res.results[c]["out"]
    return out
